# revision 6
# baseline (speedup 1.0000x reference)
"""Distributed Trainium2 kernel for nn_AdjEmbeddings (gnn_message_passing).

Strategy (8 NeuronCores, edge-sharded):
  Only ~E/NUM_USERS (~32) of the 3.2M edges match the single user_idx, so the
  only tensor that needs a full read is edge_src.  Per core (400k-edge shard):
    1. Stream the src shard [128,3125] and compare against user_idx (DVE).
    2. Block-summarize matches (blocks of 25 edges) -> [128,125] indicator.
    3. Per-partition top-3 matched-block extraction (reduce_max + clear).
    4. Indirect-DMA gather the <=3 matched blocks/partition from a host-packed
       [16000, 75] (src|dst|freq) array; re-mask; per-partition top-3 matched
       edges; unpack (dst, freq) from a packed value.
    5. Indirect-DMA gather the matched POI embedding rows; one PE matmul
       produces [1, 128+1] = (partial numerator | partial denominator).
    6. AllReduce[1,129] across the 8 cores; every core computes the epilogue
       (neigh = num/denom, concat with gathered user row, fc matmul + bias).
  Unmatched gather slots point out-of-bounds (skipped by the DMA) and carry
  weight 0, so they contribute nothing regardless of sim/HW fill behavior.
"""
import sys

if '/opt/trn_rl_repo' not in sys.path:
    sys.path.insert(0, '/opt/trn_rl_repo')

import numpy as np

NCORES = 8
E = 3_200_000
ESH = E // NCORES            # 400_000 edges per core
P = 128
FREE = ESH // P              # 3125
BLK = 25                     # edges per summary block
NBLKF = FREE // BLK          # 125 blocks per partition
NBLK = ESH // BLK            # 16000 blocks per core
TOPK = 3                     # matched blocks / edges extracted per partition
DIM = 128
NPOI = 50_000
NUSR = 100_000
BLK_SENT = 20_000            # > NBLK-1  -> OOB, skipped
POI_SENT = 60_000            # > NPOI-1  -> OOB, skipped
CLEAR = 1.0e7                # subtracted to clear extracted maxima


def _build():
    from concourse import bass, mybir
    from contextlib import ExitStack

    nc = bass.Bass(num_devices=NCORES)
    f32, i32 = mybir.dt.float32, mybir.dt.int32
    Alu = mybir.AluOpType

    src_in = nc.declare_dram_parameter("src", [P, FREE], i32, isOutput=False)
    packed_in = nc.declare_dram_parameter("packed", [NBLK, 3 * BLK], i32, isOutput=False)
    uidrep_in = nc.declare_dram_parameter("uidrep", [P, 1], f32, isOutput=False)
    uidpad_in = nc.declare_dram_parameter("uidpad", [2, 1], i32, isOutput=False)
    poi_in = nc.declare_dram_parameter("poi", [NPOI, DIM], f32, isOutput=False)
    uemb_in = nc.declare_dram_parameter("uemb", [NUSR, DIM], f32, isOutput=False)
    fcwt_in = nc.declare_dram_parameter("fcwt", [2 * DIM, DIM], f32, isOutput=False)
    fcb_in = nc.declare_dram_parameter("fcb", [1, DIM], f32, isOutput=False)
    out_ext = nc.declare_dram_parameter("out", [1, DIM], f32, isOutput=True)

    cc_in = nc.dram_tensor("cc_in", [1, DIM + 1], f32)
    cc_out = nc.dram_tensor("cc_out", [1, DIM + 1], f32, addr_space="Shared")

    es = ExitStack()

    def sb(name, shape, dt):
        return es.enter_context(nc.sbuf_tensor(name, shape, dt))

    def ps(name, shape):
        return es.enter_context(nc.psum_tensor(name, shape, f32))

    src_sb = sb('src_sb', [P, FREE], i32)
    mask_sb = sb('mask_sb', [P, FREE], f32)
    summ_sb = sb('summ_sb', [P, NBLKF], f32)
    blkio_i = sb('blkio_i', [P, NBLKF], i32)
    blkio_f = sb('blkio_f', [P, NBLKF], f32)
    cand_sb = sb('cand_sb', [P, NBLKF], f32)
    eqb_sb = sb('eqb_sb', [P, NBLKF], f32)
    mtop_sb = sb('mtop_sb', [P, TOPK], f32)
    mtmp_sb = sb('mtmp_sb', [P, TOPK], f32)
    mm_sb = sb('mm_sb', [P, TOPK], f32)
    moff_sb = sb('moff_sb', [P, TOPK], i32)
    uid_sb = sb('uid_sb', [P, 1], f32)
    upad_sb = sb('upad_sb', [2, 1], i32)
    g_sb = sb('g_sb', [P, 3 * BLK * TOPK], i32)      # gathered blocks
    mask2_sb = sb('mask2_sb', [P, BLK * TOPK], f32)
    dstf_sb = sb('dstf_sb', [P, BLK * TOPK], f32)
    freqf_sb = sb('freqf_sb', [P, BLK * TOPK], f32)
    packf_sb = sb('packf_sb', [P, BLK * TOPK], f32)
    cand2_sb = sb('cand2_sb', [P, BLK * TOPK], f32)
    eq2_sb = sb('eq2_sb', [P, BLK * TOPK], f32)
    etop_sb = sb('etop_sb', [P, TOPK], f32)
    etmp_sb = sb('etmp_sb', [P, TOPK], f32)
    em_sb = sb('em_sb', [P, TOPK], f32)
    ei_sb = sb('ei_sb', [P, TOPK], i32)
    dsti_sb = sb('dsti_sb', [P, TOPK], i32)
    freqi_sb = sb('freqi_sb', [P, TOPK], i32)
    frf_sb = sb('frf_sb', [P, TOPK], f32)
    wf_sb = sb('wf_sb', [P, TOPK], f32)
    dstf2_sb = sb('dstf2_sb', [P, TOPK], f32)
    dstoff_sb = sb('dstoff_sb', [P, TOPK], i32)
    paug_sb = sb('paug_sb', [P, TOPK * (DIM + 1)], f32)
    u2_sb = sb('u2_sb', [2, DIM], f32)
    ucol_sb = sb('ucol_sb', [P, 1], f32)
    ncol_sb = sb('ncol_sb', [P, 1], f32)
    nd_sb = sb('nd_sb', [1, DIM + 1], f32)
    red_sb = sb('red_sb', [1, DIM + 1], f32)
    gg_sb = sb('gg_sb', [1, 1], f32)
    onem_sb = sb('onem_sb', [1, 1], f32)
    saf_sb = sb('saf_sb', [1, 1], f32)
    rs_sb = sb('rs_sb', [1, 1], f32)
    scale_sb = sb('scale_sb', [1, 1], f32)
    neigh_sb = sb('neigh_sb', [1, DIM], f32)
    fcw1_sb = sb('fcw1_sb', [P, DIM], f32)
    fcw2_sb = sb('fcw2_sb', [P, DIM], f32)
    fcb_sb = sb('fcb_sb', [1, DIM], f32)
    out_sb = sb('out_sb', [1, DIM], f32)
    ones11_sb = sb('ones11_sb', [1, 1], f32)

    psum_nd = ps('psum_nd', [1, DIM + 1])
    psum_t1 = ps('psum_t1', [P, 1])
    psum_t2 = ps('psum_t2', [P, 1])
    psum_fc = ps('psum_fc', [1, DIM])

    CH0 = 1575                      # chunk split (multiple of BLK)
    NB0 = CH0 // BLK                # 63 blocks

    MK = {}
    with (
        nc.semaphore("vq") as vq,
        nc.semaphore("sS0") as sS0,
        nc.semaphore("sS1") as sS1,
        nc.semaphore("sC") as sC,
        nc.semaphore("sW") as sW,
        nc.semaphore("sGp") as sGp,     # gpsimd compute progress (iota/memsets)
        nc.semaphore("sGB") as sGB,     # block gathers
        nc.semaphore("sGP") as sGP,     # poi gathers
        nc.semaphore("sU") as sU,       # user row gather
        nc.semaphore("sPE") as sPE,
        nc.semaphore("sSC") as sSC,
        nc.semaphore("sCCI") as sCCI,
        nc.semaphore("sCC") as sCC,
        nc.semaphore("sRED") as sRED,
        nc.Block() as block,
    ):
        @block.vector
        def _(vector):
            v = nc.vector
            nv = [0]

            def step(inst):
                inst.then_inc(vq, 1)
                nv[0] += 1
                vector.wait_ge(vq, nv[0])
                return nv[0]

            step(v.memset(ones11_sb[:], 1.0))
            MK['ones11'] = nv[0]
            vector.wait_ge(sGp, 1)
            step(v.tensor_copy(out=blkio_f[:], in_=blkio_i[:]))
            vector.wait_ge(sC, 48)
            vector.wait_ge(sS0, 16)
            step(v.tensor_scalar(out=mask_sb[:, 0:CH0], in0=src_sb[:, 0:CH0],
                                 scalar1=uid_sb[:, :1], scalar2=None, op0=Alu.is_equal))
            step(v.tensor_reduce(
                out=summ_sb[:, 0:NB0],
                in_=mask_sb[:, 0:CH0].rearrange("p (b w) -> p b w", w=BLK),
                axis=mybir.AxisListType.X, op=Alu.max))
            vector.wait_ge(sS1, 16)
            step(v.tensor_scalar(out=mask_sb[:, CH0:FREE], in0=src_sb[:, CH0:FREE],
                                 scalar1=uid_sb[:, :1], scalar2=None, op0=Alu.is_equal))
            step(v.tensor_reduce(
                out=summ_sb[:, NB0:NBLKF],
                in_=mask_sb[:, CH0:FREE].rearrange("p (b w) -> p b w", w=BLK),
                axis=mybir.AxisListType.X, op=Alu.max))
            # cand = summ * (blkid+1) - 1
            step(v.tensor_tensor(out=cand_sb[:], in0=summ_sb[:], in1=blkio_f[:], op=Alu.mult))
            step(v.tensor_scalar_add(out=cand_sb[:], in0=cand_sb[:], scalar1=-1.0))
            for j in range(TOPK):
                step(v.tensor_reduce(out=mtop_sb[:, j:j + 1], in_=cand_sb[:],
                                     axis=mybir.AxisListType.X, op=Alu.max))
                if j < TOPK - 1:
                    step(v.tensor_scalar(out=eqb_sb[:], in0=cand_sb[:],
                                         scalar1=mtop_sb[:, j:j + 1], scalar2=None,
                                         op0=Alu.is_equal))
                    step(v.tensor_scalar(out=eqb_sb[:], in0=eqb_sb[:], scalar1=CLEAR,
                                         scalar2=None, op0=Alu.mult))
                    step(v.tensor_tensor(out=cand_sb[:], in0=cand_sb[:], in1=eqb_sb[:],
                                         op=Alu.subtract))
            # mtop holds blkid (>=0) for matched, <= -1 otherwise
            # moff = matched ? blkid : BLK_SENT
            step(v.tensor_scalar(out=mm_sb[:], in0=mtop_sb[:], scalar1=0.0,
                                 scalar2=None, op0=Alu.is_ge))
            step(v.tensor_scalar(out=mtop_sb[:], in0=mtop_sb[:], scalar1=-1.0,
                                 scalar2=None, op0=Alu.max))
            step(v.tensor_scalar(out=mtmp_sb[:], in0=mtop_sb[:],
                                 scalar1=-float(BLK_SENT), scalar2=None, op0=Alu.add))
            step(v.tensor_tensor(out=mtmp_sb[:], in0=mtmp_sb[:], in1=mm_sb[:], op=Alu.mult))
            step(v.tensor_scalar_add(out=mtmp_sb[:], in0=mtmp_sb[:], scalar1=float(BLK_SENT)))
            step(v.tensor_copy(out=moff_sb[:], in_=mtmp_sb[:]))
            MK['moff'] = nv[0]
            # ---- level 2: gathered blocks -> matched edges
            vector.wait_ge(sGB, 48)
            g3 = g_sb[:].rearrange("p (j c) -> p j c", c=3 * BLK)
            m23 = mask2_sb[:].rearrange("p (j c) -> p j c", c=BLK)
            d3 = dstf_sb[:].rearrange("p (j c) -> p j c", c=BLK)
            f3 = freqf_sb[:].rearrange("p (j c) -> p j c", c=BLK)
            step(v.tensor_scalar(out=m23, in0=g3[:, :, 0:BLK], scalar1=uid_sb[:, :1],
                                 scalar2=None, op0=Alu.is_equal))
            step(v.tensor_copy(out=d3, in_=g3[:, :, BLK:2 * BLK]))
            step(v.tensor_copy(out=f3, in_=g3[:, :, 2 * BLK:3 * BLK]))
            step(v.tensor_scalar(out=packf_sb[:], in0=dstf_sb[:], scalar1=64.0,
                                 scalar2=1.0, op0=Alu.mult, op1=Alu.add))
            step(v.tensor_tensor(out=packf_sb[:], in0=packf_sb[:], in1=freqf_sb[:], op=Alu.add))
            step(v.tensor_tensor(out=cand2_sb[:], in0=packf_sb[:], in1=mask2_sb[:], op=Alu.mult))
            step(v.tensor_scalar_add(out=cand2_sb[:], in0=cand2_sb[:], scalar1=-1.0))
            for j in range(TOPK):
                step(v.tensor_reduce(out=etop_sb[:, j:j + 1], in_=cand2_sb[:],
                                     axis=mybir.AxisListType.X, op=Alu.max))
                if j < TOPK - 1:
                    step(v.tensor_scalar(out=eq2_sb[:], in0=cand2_sb[:],
                                         scalar1=etop_sb[:, j:j + 1], scalar2=None,
                                         op0=Alu.is_equal))
                    step(v.tensor_scalar(out=eq2_sb[:], in0=eq2_sb[:], scalar1=CLEAR,
                                         scalar2=None, op0=Alu.mult))
                    step(v.tensor_tensor(out=cand2_sb[:], in0=cand2_sb[:], in1=eq2_sb[:],
                                         op=Alu.subtract))
            step(v.tensor_scalar(out=etop_sb[:], in0=etop_sb[:], scalar1=-1.0,
                                 scalar2=None, op0=Alu.max))
            step(v.tensor_scalar(out=em_sb[:], in0=etop_sb[:], scalar1=0.0,
                                 scalar2=None, op0=Alu.is_ge))
            step(v.tensor_copy(out=ei_sb[:], in_=etop_sb[:]))
            step(v.tensor_scalar(out=dsti_sb[:], in0=ei_sb[:], scalar1=6, scalar2=None,
                                 op0=Alu.arith_shift_right))
            step(v.tensor_scalar(out=freqi_sb[:], in0=ei_sb[:], scalar1=63, scalar2=None,
                                 op0=Alu.bitwise_and))
            step(v.tensor_copy(out=frf_sb[:], in_=freqi_sb[:]))
            step(v.tensor_tensor(out=wf_sb[:], in0=frf_sb[:], in1=em_sb[:], op=Alu.mult))
            step(v.tensor_copy(out=dstf2_sb[:], in_=dsti_sb[:]))
            step(v.tensor_scalar_add(out=dstf2_sb[:], in0=dstf2_sb[:], scalar1=-float(POI_SENT)))
            step(v.tensor_tensor(out=dstf2_sb[:], in0=dstf2_sb[:], in1=em_sb[:], op=Alu.mult))
            step(v.tensor_scalar_add(out=dstf2_sb[:], in0=dstf2_sb[:], scalar1=float(POI_SENT)))
            step(v.tensor_copy(out=dstoff_sb[:], in_=dstf2_sb[:]))
            MK['dstoff'] = nv[0]
            # ---- epilogue (after allreduce round trip)
            vector.wait_ge(sRED, 16)
            step(v.tensor_scalar(out=gg_sb[:], in0=red_sb[0:1, DIM:DIM + 1], scalar1=0.0,
                                 scalar2=None, op0=Alu.is_gt))
            step(v.tensor_scalar(out=onem_sb[:], in0=gg_sb[:], scalar1=-1.0, scalar2=1.0,
                                 op0=Alu.mult, op1=Alu.add))
            step(v.tensor_tensor(out=saf_sb[:], in0=red_sb[0:1, DIM:DIM + 1], in1=gg_sb[:],
                                 op=Alu.mult))
            step(v.tensor_tensor(out=saf_sb[:], in0=saf_sb[:], in1=onem_sb[:], op=Alu.add))
            step(v.reciprocal(out=rs_sb[:], in_=saf_sb[:]))
            step(v.tensor_tensor(out=scale_sb[:], in0=rs_sb[:], in1=gg_sb[:], op=Alu.mult))
            step(v.tensor_tensor(out=neigh_sb[:], in0=red_sb[0:1, 0:DIM],
                                 in1=scale_sb[0:1, 0:1].to_broadcast([1, DIM]),
                                 op=Alu.mult))
            MK['neigh'] = nv[0]
            vector.wait_ge(sPE, 4)
            step(v.tensor_tensor(out=out_sb[:], in0=psum_fc[:], in1=fcb_sb[:],
                                 op=Alu.add))
            MK['out'] = nv[0]

        @block.sync
        def _(sync):
            sync.dma_start(out=src_sb[:, 0:CH0], in_=src_in[:, 0:CH0]).then_inc(sS0, 16)
            sync.dma_start(out=src_sb[:, CH0:FREE], in_=src_in[:, CH0:FREE]).then_inc(sS1, 16)
            sync.dma_start(out=uid_sb[:], in_=uidrep_in[:]).then_inc(sC, 16)
            sync.dma_start(out=upad_sb[:], in_=uidpad_in[:]).then_inc(sC, 16)
            sync.dma_start(out=fcb_sb[:], in_=fcb_in[:]).then_inc(sC, 16)
            sync.dma_start(out=fcw1_sb[:], in_=fcwt_in[0:DIM, :]).then_inc(sW, 16)
            sync.dma_start(out=fcw2_sb[:], in_=fcwt_in[DIM:2 * DIM, :]).then_inc(sW, 16)
            sync.wait_ge(sSC, 1)     # nd_sb ready
            sync.dma_start(out=cc_in[:], in_=nd_sb[:]).then_inc(sCCI, 16)
            sync.wait_ge(sCC, 1)     # collective done
            sync.dma_start(out=red_sb[:], in_=cc_out[:]).then_inc(sRED, 16)
            sync.wait_ge(vq, MK['out'])
            sync.dma_start(out=out_ext[:], in_=out_sb[:]).then_inc(sS0, 16)

        @block.gpsimd
        def _(gpsimd):
            gpsimd.iota(blkio_i[:], pattern=[[1, NBLKF]], base=1,
                        channel_multiplier=NBLKF).then_inc(sGp, 1)
            gpsimd.memset(g_sb[:], -1).then_inc(sGp, 1)
            gpsimd.memset(paug_sb[:], 0.0).then_inc(sGp, 1)
            gpsimd.memset(
                paug_sb[:].rearrange("p (j c) -> p j c", c=DIM + 1)[:, :, DIM:DIM + 1],
                1.0).then_inc(sGp, 1)
            gpsimd.wait_ge(sC, 48)
            gpsimd.indirect_dma_start(
                out=u2_sb[:], out_offset=None, in_=uemb_in[:],
                in_offset=bass.IndirectOffsetOnAxis(ap=upad_sb[:, :1], axis=0),
                bounds_check=NUSR - 1, oob_is_err=False).then_inc(sU, 16)
            gpsimd.wait_ge(sGp, 4)   # own memsets complete
            gpsimd.wait_ge(vq, MK['moff'])
            for j in range(TOPK):
                gpsimd.indirect_dma_start(
                    out=g_sb[:, j * 3 * BLK:(j + 1) * 3 * BLK], out_offset=None,
                    in_=packed_in[:],
                    in_offset=bass.IndirectOffsetOnAxis(ap=moff_sb[:, j:j + 1], axis=0),
                    bounds_check=NBLK - 1, oob_is_err=False).then_inc(sGB, 16)
            gpsimd.wait_ge(vq, MK['dstoff'])
            for j in range(TOPK):
                gpsimd.indirect_dma_start(
                    out=paug_sb[:, j * (DIM + 1):j * (DIM + 1) + DIM], out_offset=None,
                    in_=poi_in[:],
                    in_offset=bass.IndirectOffsetOnAxis(ap=dstoff_sb[:, j:j + 1], axis=0),
                    bounds_check=NPOI - 1, oob_is_err=False).then_inc(sGP, 16)
            gpsimd.wait_ge(sCCI, 16)
            gpsimd.collective_compute(
                "AllReduce", Alu.add, replica_groups=[list(range(NCORES))],
                ins=[cc_in[:]], outs=[cc_out[:]]).then_inc(sCC, 1)

        @block.tensor
        def _(tensor):
            tensor.wait_ge(vq, MK['dstoff'])
            tensor.wait_ge(sGP, 48)
            for j in range(TOPK):
                mm = nc.tensor.matmul(
                    out=psum_nd[:], lhsT=wf_sb[:, j:j + 1],
                    rhs=paug_sb[:, j * (DIM + 1):(j + 1) * (DIM + 1)],
                    start=(j == 0), stop=(j == TOPK - 1))
            mm.then_inc(sPE, 1)
            tensor.wait_ge(sU, 16)
            tensor.wait_ge(vq, MK['ones11'])
            nc.tensor.transpose(out=psum_t1[:], in_=u2_sb[0:1, :],
                                identity=ones11_sb[:]).then_inc(sPE, 1)
            tensor.wait_ge(vq, MK['neigh'])
            nc.tensor.transpose(out=psum_t2[:], in_=neigh_sb[:],
                                identity=ones11_sb[:]).then_inc(sPE, 1)
            tensor.wait_ge(sSC, 3)
            tensor.wait_ge(sW, 32)
            nc.tensor.matmul(out=psum_fc[:], lhsT=ucol_sb[:], rhs=fcw1_sb[:],
                             start=True, stop=False)
            nc.tensor.matmul(out=psum_fc[:], lhsT=ncol_sb[:], rhs=fcw2_sb[:],
                             start=False, stop=True).then_inc(sPE, 1)

        @block.scalar
        def _(scalar):
            scalar.wait_ge(sPE, 1)
            nc.scalar.copy(out=nd_sb[:], in_=psum_nd[:]).then_inc(sSC, 1)
            scalar.wait_ge(sPE, 2)
            nc.scalar.copy(out=ucol_sb[:], in_=psum_t1[:]).then_inc(sSC, 1)
            scalar.wait_ge(sPE, 3)
            nc.scalar.copy(out=ncol_sb[:], in_=psum_t2[:]).then_inc(sSC, 1)

    es.close()
    return nc


_BUILT = None


def _get_nc():
    global _BUILT
    if _BUILT is None:
        _BUILT = _build()
    return _BUILT


def kernel(**inputs):
    from concourse.bass_utils import run_bass_kernel_spmd

    user_idx = np.asarray(inputs["user_idx"]).astype(np.int32)
    poi = np.ascontiguousarray(np.asarray(inputs["poi_embeddings"], dtype=np.float32))
    src = np.asarray(inputs["edge_src"]).astype(np.int32)
    dst = np.asarray(inputs["edge_dst"]).astype(np.int32)
    freq = np.asarray(inputs["edge_freq"]).astype(np.int32)
    uemb = np.ascontiguousarray(np.asarray(inputs["user_emb"], dtype=np.float32))
    fc_w = np.asarray(inputs["fc_w"], dtype=np.float32)
    fc_b = np.asarray(inputs["fc_b"], dtype=np.float32)

    uid = int(user_idx[0])
    uidrep = np.full((P, 1), float(uid), np.float32)
    uidpad = np.full((2, 1), uid, np.int32)
    fcwt = np.ascontiguousarray(fc_w.T)
    fcb = fc_b.reshape(1, DIM)

    # safety: the static graph extracts at most TOPK matched blocks and TOPK
    # matched edges per partition; verify the actual data fits (P(fail) ~ 1e-5
    # for this generator; fail loudly rather than return a wrong answer).
    m = src == uid
    mpart = m.reshape(NCORES * P, FREE)
    epp = mpart.sum(1)
    bpp = mpart.reshape(NCORES * P, NBLKF, BLK).any(2).sum(1)
    assert epp.max() <= TOPK, f"edges/partition {epp.max()} > {TOPK}"
    assert bpp.max() <= TOPK, f"blocks/partition {bpp.max()} > {TOPK}"
    # no duplicate (dst,freq) among matched edges within one partition
    packs = (dst.astype(np.int64) * 64 + freq)[:].reshape(NCORES * P, FREE)
    for prow in np.nonzero(epp > 1)[0]:
        vals = packs[prow][mpart[prow]]
        assert len(set(vals.tolist())) == len(vals), "duplicate (dst,freq) in partition"

    nc = _get_nc()
    in_maps = []
    for c in range(NCORES):
        sl = slice(c * ESH, (c + 1) * ESH)
        packed = np.concatenate(
            [src[sl].reshape(NBLK, BLK), dst[sl].reshape(NBLK, BLK),
             freq[sl].reshape(NBLK, BLK)], axis=1)
        in_maps.append({
            "src": np.ascontiguousarray(src[sl].reshape(P, FREE)),
            "packed": np.ascontiguousarray(packed),
            "uidrep": uidrep, "uidpad": uidpad,
            "poi": poi, "uemb": uemb, "fcwt": fcwt, "fcb": fcb,
        })
    res = run_bass_kernel_spmd(nc, in_maps, list(range(NCORES)))
    return np.asarray(res.results[0]["out"], dtype=np.float32)


# revision 8
# speedup vs baseline: 1.2305x; 1.2305x over previous
"""Distributed Trainium2 kernel for nn_AdjEmbeddings (gnn_message_passing).

Strategy (8 NeuronCores, edge-sharded):
  Only ~E/NUM_USERS (~32) of the 3.2M edges match the single user_idx, so the
  only tensor that needs a full read is edge_src.  Per core (400k-edge shard):
    1. Stream the src shard [128,3125] and compare against user_idx (DVE).
    2. Block-summarize matches (blocks of 25 edges) -> [128,125] indicator.
    3. Per-partition top-2 matched-block extraction (reduce_max + clear).
    4. Indirect-DMA gather the <=2 matched blocks/partition from a host-packed
       [16000, 75] (src|dst|freq) array; re-mask; per-partition top-2 matched
       edges; unpack (dst, freq) from a packed value dst*64+freq.
    5. Indirect-DMA gather the matched POI embedding rows; PE matmuls produce
       [1, 128+1] = (partial numerator | partial denominator).
    6. AllGather[8,129] across the 8 cores; every core reduces the partials
       locally and computes the epilogue (neigh = num/max(den,1), fc matmuls).
  Unmatched gather slots point out-of-bounds (skipped by the DMA) and carry
  weight 0, so they contribute nothing regardless of sim/HW fill behavior.
  NOTE: same-engine RAW hazards are real on this HW -- every dependent DVE op
  is serialized through the vq semaphore.
"""
import sys

if '/opt/trn_rl_repo' not in sys.path:
    sys.path.insert(0, '/opt/trn_rl_repo')

import numpy as np

NCORES = 8
E = 3_200_000
ESH = E // NCORES            # 400_000 edges per core
P = 128
FREE = ESH // P              # 3125
BLK = 25                     # edges per summary block
NBLKF = FREE // BLK          # 125 blocks per partition
NBLK = ESH // BLK            # 16000 blocks per core
TOPK = 2                     # matched blocks / edges extracted per partition
DIM = 128
NPOI = 50_000
NUSR = 100_000
BLK_SENT = 20_000            # > NBLK-1  -> OOB, skipped
POI_SENT = 60_000            # > NPOI-1  -> OOB, skipped
CLEAR = 1.0e7                # subtracted to clear extracted maxima
CH0 = 1575                   # stream chunk split (multiple of BLK)
NB0 = CH0 // BLK


def _build():
    from concourse import bass, mybir
    from contextlib import ExitStack

    nc = bass.Bass(num_devices=NCORES)
    f32, i32 = mybir.dt.float32, mybir.dt.int32
    Alu = mybir.AluOpType
    X = mybir.AxisListType.X

    src_in = nc.declare_dram_parameter("src", [P, FREE], i32, isOutput=False)
    packed_in = nc.declare_dram_parameter("packed", [NBLK, 3 * BLK], i32, isOutput=False)
    uidrep_in = nc.declare_dram_parameter("uidrep", [P, 1], f32, isOutput=False)
    uidpad_in = nc.declare_dram_parameter("uidpad", [2, 1], i32, isOutput=False)
    blkio_in = nc.declare_dram_parameter("blkio", [P, NBLKF], f32, isOutput=False)
    poi_in = nc.declare_dram_parameter("poi", [NPOI, DIM], f32, isOutput=False)
    uemb_in = nc.declare_dram_parameter("uemb", [NUSR, DIM], f32, isOutput=False)
    fcwt_in = nc.declare_dram_parameter("fcwt", [2 * DIM, DIM], f32, isOutput=False)
    fcb_in = nc.declare_dram_parameter("fcb", [1, DIM], f32, isOutput=False)
    out_ext = nc.declare_dram_parameter("out", [1, DIM], f32, isOutput=True)

    cc_in = nc.dram_tensor("cc_in", [1, DIM + 1], f32)
    cc_ag = nc.dram_tensor("cc_ag", [NCORES, DIM + 1], f32, addr_space="Shared")

    es = ExitStack()

    def sb(name, shape, dt):
        return es.enter_context(nc.sbuf_tensor(name, shape, dt))

    def ps(name, shape):
        return es.enter_context(nc.psum_tensor(name, shape, f32))

    src_sb = sb('src_sb', [P, FREE], i32)
    mask_sb = sb('mask_sb', [P, FREE], f32)
    summ_sb = sb('summ_sb', [P, NBLKF], f32)
    blkio_sb = sb('blkio_sb', [P, NBLKF], f32)
    cand_sb = sb('cand_sb', [P, NBLKF], f32)
    eqb_sb = sb('eqb_sb', [P, NBLKF], f32)
    mtop_sb = sb('mtop_sb', [P, TOPK], f32)
    mm_sb = sb('mm_sb', [P, TOPK], f32)
    mtmp_sb = sb('mtmp_sb', [P, TOPK], f32)
    moff_sb = sb('moff_sb', [P, TOPK], i32)
    uid_sb = sb('uid_sb', [P, 1], f32)
    upad_sb = sb('upad_sb', [2, 1], i32)
    warmoff_sb = sb('warmoff_sb', [2, 1], i32)
    warm_sb = sb('warm_sb', [2, 3 * BLK], i32)
    g_sb = sb('g_sb', [P, 3 * BLK * TOPK], i32)
    mask2_sb = sb('mask2_sb', [P, BLK * TOPK], f32)
    dstf_sb = sb('dstf_sb', [P, BLK * TOPK], f32)
    freqf_sb = sb('freqf_sb', [P, BLK * TOPK], f32)
    packf_sb = sb('packf_sb', [P, BLK * TOPK], f32)
    cand2_sb = sb('cand2_sb', [P, BLK * TOPK], f32)
    eq2_sb = sb('eq2_sb', [P, BLK * TOPK], f32)
    etop_sb = sb('etop_sb', [P, TOPK], f32)
    em_sb = sb('em_sb', [P, TOPK], f32)
    ei_sb = sb('ei_sb', [P, TOPK], i32)
    dsti_sb = sb('dsti_sb', [P, TOPK], i32)
    freqi_sb = sb('freqi_sb', [P, TOPK], i32)
    frf_sb = sb('frf_sb', [P, TOPK], f32)
    wf_sb = sb('wf_sb', [P, TOPK], f32)
    dstf2_sb = sb('dstf2_sb', [P, TOPK], f32)
    dstt_sb = sb('dstt_sb', [P, TOPK], f32)
    dstoff_sb = sb('dstoff_sb', [P, TOPK], i32)
    paug_sb = sb('paug_sb', [P, TOPK * (DIM + 1)], f32)
    u2_sb = sb('u2_sb', [2, DIM], f32)
    ucol_sb = sb('ucol_sb', [P, 1], f32)
    ncol_sb = sb('ncol_sb', [P, 1], f32)
    nd_sb = sb('nd_sb', [1, DIM + 1], f32)
    cc8_sb = sb('cc8_sb', [NCORES, DIM + 1], f32)
    ones8_sb = sb('ones8_sb', [NCORES, 1], f32)
    saf_sb = sb('saf_sb', [1, 1], f32)
    rs_sb = sb('rs_sb', [1, 1], f32)
    t1_sb = sb('t1_sb', [1, DIM], f32)
    t2_sb = sb('t2_sb', [1, DIM], f32)
    fcw1_sb = sb('fcw1_sb', [P, DIM], f32)
    fcw2_sb = sb('fcw2_sb', [P, DIM], f32)
    fcb_sb = sb('fcb_sb', [1, DIM], f32)
    out_sb = sb('out_sb', [1, DIM], f32)
    ones11_sb = sb('ones11_sb', [1, 1], f32)

    psum_t1 = ps('psum_t1', [P, 1])
    psum_fc1 = ps('psum_fc1', [1, DIM])
    psum_nd = ps('psum_nd', [1, DIM + 1])
    psum_fc2 = ps('psum_fc2', [1, DIM])
    psum_nc = ps('psum_nc', [P, 1])
    psum_den = ps('psum_den', [1, 1])

    MK = {}
    with (
        nc.semaphore("vq") as vq,
        nc.semaphore("sS0") as sS0,
        nc.semaphore("sS1") as sS1,
        nc.semaphore("sC") as sC,
        nc.semaphore("sG") as sG,
        nc.semaphore("sPE") as sPE,
        nc.semaphore("sCCI") as sCCI,
        nc.semaphore("sCC") as sCC,
        nc.semaphore("sRED") as sRED,
        nc.Block() as block,
    ):
        @block.vector
        def _(vector):
            v = nc.vector
            nv = [0]

            def step(inst, wait=True):
                inst.then_inc(vq, 1)
                nv[0] += 1
                # serialize same-engine RAW hazards; independent ops may skip
                if wait:
                    vector.wait_ge(vq, nv[0])
                return nv[0]

            # independent setup (no internal deps -> no waits between them)
            step(v.memset(warmoff_sb[:], 0), wait=False)
            step(v.memset(ones11_sb[:], 1.0), wait=False)
            step(v.memset(g_sb[:], -1), wait=False)
            step(v.memset(paug_sb[:], 0.0), wait=False)
            step(v.memset(
                paug_sb[:].rearrange("p (j c) -> p j c", c=DIM + 1)[:, :, DIM:DIM + 1],
                1.0), wait=False)
            step(v.memset(ones8_sb[:], 1.0), wait=False)
            MK['setup'] = nv[0]
            vector.wait_ge(vq, nv[0])
            vector.wait_ge(sC, 80)
            vector.wait_ge(sS0, 32)     # blkio + src chunk0
            step(v.tensor_scalar(out=mask_sb[:, 0:CH0], in0=src_sb[:, 0:CH0],
                                 scalar1=uid_sb[:, :1], scalar2=None, op0=Alu.is_equal))
            step(v.tensor_reduce(
                out=summ_sb[:, 0:NB0],
                in_=mask_sb[:, 0:CH0].rearrange("p (b w) -> p b w", w=BLK),
                axis=X, op=Alu.max))
            vector.wait_ge(sS1, 16)
            step(v.tensor_scalar(out=mask_sb[:, CH0:FREE], in0=src_sb[:, CH0:FREE],
                                 scalar1=uid_sb[:, :1], scalar2=None, op0=Alu.is_equal))
            step(v.tensor_reduce(
                out=summ_sb[:, NB0:NBLKF],
                in_=mask_sb[:, CH0:FREE].rearrange("p (b w) -> p b w", w=BLK),
                axis=X, op=Alu.max))
            # cand = summ * (blkid+1) - 1   (blkio holds blkid+1)
            step(v.tensor_tensor(out=cand_sb[:], in0=summ_sb[:], in1=blkio_sb[:],
                                 op=Alu.mult))
            step(v.tensor_scalar_add(out=cand_sb[:], in0=cand_sb[:], scalar1=-1.0))
            # top-2 blocks per partition
            step(v.tensor_reduce(out=mtop_sb[:, 0:1], in_=cand_sb[:], axis=X, op=Alu.max))
            step(v.tensor_scalar(out=eqb_sb[:], in0=cand_sb[:],
                                 scalar1=mtop_sb[:, 0:1], scalar2=CLEAR,
                                 op0=Alu.is_equal, op1=Alu.mult))
            step(v.tensor_tensor(out=cand_sb[:], in0=cand_sb[:], in1=eqb_sb[:],
                                 op=Alu.subtract))
            step(v.tensor_reduce(out=mtop_sb[:, 1:2], in_=cand_sb[:], axis=X, op=Alu.max))
            # moff = matched ? blkid : BLK_SENT  (mtop holds blkid, or < 0)
            step(v.tensor_scalar(out=mm_sb[:], in0=mtop_sb[:], scalar1=0.0,
                                 scalar2=None, op0=Alu.is_ge))
            step(v.scalar_tensor_tensor(out=mtmp_sb[:], in0=mtop_sb[:],
                                        scalar=-float(BLK_SENT), in1=mm_sb[:],
                                        op0=Alu.add, op1=Alu.mult))
            step(v.tensor_scalar(out=moff_sb[:], in0=mtmp_sb[:],
                                 scalar1=float(BLK_SENT), scalar2=None, op0=Alu.add))
            MK['moff'] = nv[0]
            # ---- level 2: gathered blocks -> matched edges
            vector.wait_ge(sG, 64)          # warm(16)+u(16)+2 block gathers
            g3 = g_sb[:].rearrange("p (j c) -> p j c", c=3 * BLK)
            m23 = mask2_sb[:].rearrange("p (j c) -> p j c", c=BLK)
            d3 = dstf_sb[:].rearrange("p (j c) -> p j c", c=BLK)
            f3 = freqf_sb[:].rearrange("p (j c) -> p j c", c=BLK)
            step(v.tensor_scalar(out=m23, in0=g3[:, :, 0:BLK], scalar1=uid_sb[:, :1],
                                 scalar2=None, op0=Alu.is_equal), wait=False)
            step(v.tensor_copy(out=d3, in_=g3[:, :, BLK:2 * BLK]), wait=False)
            step(v.tensor_copy(out=f3, in_=g3[:, :, 2 * BLK:3 * BLK]))
            # packf = dst*64 + freq ; cand2 = (packf+1)*mask2 - 1
            step(v.scalar_tensor_tensor(out=packf_sb[:], in0=dstf_sb[:], scalar=64.0,
                                        in1=freqf_sb[:], op0=Alu.mult, op1=Alu.add))
            step(v.scalar_tensor_tensor(out=cand2_sb[:], in0=packf_sb[:], scalar=1.0,
                                        in1=mask2_sb[:], op0=Alu.add, op1=Alu.mult))
            step(v.tensor_scalar_add(out=cand2_sb[:], in0=cand2_sb[:], scalar1=-1.0))
            step(v.tensor_reduce(out=etop_sb[:, 0:1], in_=cand2_sb[:], axis=X, op=Alu.max))
            step(v.tensor_scalar(out=eq2_sb[:], in0=cand2_sb[:],
                                 scalar1=etop_sb[:, 0:1], scalar2=CLEAR,
                                 op0=Alu.is_equal, op1=Alu.mult))
            step(v.tensor_tensor(out=cand2_sb[:], in0=cand2_sb[:], in1=eq2_sb[:],
                                 op=Alu.subtract))
            step(v.tensor_reduce(out=etop_sb[:, 1:2], in_=cand2_sb[:], axis=X, op=Alu.max))
            # unpack: etop = dst*64+freq (>=64) matched, else < 0
            step(v.tensor_scalar(out=em_sb[:], in0=etop_sb[:], scalar1=0.0,
                                 scalar2=None, op0=Alu.is_ge))
            step(v.tensor_copy(out=ei_sb[:], in_=etop_sb[:]))
            step(v.tensor_scalar(out=dsti_sb[:], in0=ei_sb[:], scalar1=6, scalar2=None,
                                 op0=Alu.arith_shift_right), wait=False)
            step(v.tensor_scalar(out=freqi_sb[:], in0=ei_sb[:], scalar1=63, scalar2=None,
                                 op0=Alu.bitwise_and))
            step(v.tensor_copy(out=frf_sb[:], in_=freqi_sb[:]), wait=False)
            step(v.tensor_copy(out=dstf2_sb[:], in_=dsti_sb[:]))
            step(v.tensor_tensor(out=wf_sb[:], in0=frf_sb[:], in1=em_sb[:],
                                 op=Alu.mult), wait=False)
            step(v.scalar_tensor_tensor(out=dstt_sb[:], in0=dstf2_sb[:],
                                        scalar=-float(POI_SENT), in1=em_sb[:],
                                        op0=Alu.add, op1=Alu.mult))
            step(v.tensor_scalar(out=dstoff_sb[:], in0=dstt_sb[:],
                                 scalar1=float(POI_SENT), scalar2=None, op0=Alu.add))
            MK['dstoff'] = nv[0]
            # u column for the fc matmul (PE transposed it into psum_t1)
            vector.wait_ge(sPE, 1)
            step(v.tensor_copy(out=ucol_sb[:], in_=psum_t1[:]))
            MK['ucol'] = nv[0]
            # partials out for the collective
            vector.wait_ge(sPE, 3)
            step(v.tensor_copy(out=nd_sb[:], in_=psum_nd[:]))
            MK['nd'] = nv[0]
            # ---- after allgather: PE summed the partials into psum_nc/psum_den
            vector.wait_ge(sPE, 4)
            step(v.tensor_copy(out=ncol_sb[:], in_=psum_nc[:]), wait=False)
            # den is 0 (no matches anywhere -> num==0) or >= 1
            step(v.tensor_scalar(out=saf_sb[:], in0=psum_den[:], scalar1=1.0,
                                 scalar2=None, op0=Alu.max))
            MK['ncol'] = nv[0]
            step(v.reciprocal(out=rs_sb[:], in_=saf_sb[:]))
            MK['rs'] = nv[0]
            vector.wait_ge(sPE, 5)
            step(v.tensor_scalar(out=t1_sb[:], in0=psum_fc2[:], scalar1=rs_sb[0:1, :1],
                                 scalar2=None, op0=Alu.mult))
            step(v.tensor_tensor(out=t2_sb[:], in0=t1_sb[:], in1=psum_fc1[:], op=Alu.add))
            step(v.tensor_tensor(out=out_sb[:], in0=t2_sb[:], in1=fcb_sb[:], op=Alu.add))
            MK['out'] = nv[0]

        @block.sync
        def _(sync):
            sync.dma_start(out=uid_sb[:], in_=uidrep_in[:]).then_inc(sC, 16)
            sync.dma_start(out=upad_sb[:], in_=uidpad_in[:]).then_inc(sC, 16)
            sync.dma_start(out=fcb_sb[:], in_=fcb_in[:]).then_inc(sC, 16)
            sync.dma_start(out=fcw1_sb[:], in_=fcwt_in[0:DIM, :]).then_inc(sC, 16)
            sync.dma_start(out=fcw2_sb[:], in_=fcwt_in[DIM:2 * DIM, :]).then_inc(sC, 16)
            sync.dma_start(out=blkio_sb[:], in_=blkio_in[:]).then_inc(sS0, 16)
            sync.wait_ge(vq, MK['nd'])
            sync.dma_start(out=cc_in[:], in_=nd_sb[:]).then_inc(sCCI, 16)
            sync.wait_ge(sCC, 1)
            sync.dma_start(out=cc8_sb[:], in_=cc_ag[:]).then_inc(sRED, 16)
            sync.wait_ge(vq, MK['out'])
            sync.dma_start(out=out_ext[:], in_=out_sb[:]).then_inc(sS0, 16)

        @block.scalar
        def _(scalar):
            # second HWDGE ring: the big src stream
            scalar.dma_start(out=src_sb[:, 0:CH0], in_=src_in[:, 0:CH0]).then_inc(sS0, 16)
            scalar.dma_start(out=src_sb[:, CH0:FREE], in_=src_in[:, CH0:FREE]).then_inc(sS1, 16)

        @block.gpsimd
        def _(gpsimd):
            # warmup: pulls the indirect-DMA ucode load off the critical path
            gpsimd.wait_ge(vq, MK['setup'])
            gpsimd.indirect_dma_start(
                out=warm_sb[:], out_offset=None, in_=packed_in[:],
                in_offset=bass.IndirectOffsetOnAxis(ap=warmoff_sb[:, :1], axis=0),
                bounds_check=NBLK - 1, oob_is_err=False).then_inc(sG, 16)
            gpsimd.wait_ge(sC, 80)
            gpsimd.indirect_dma_start(
                out=u2_sb[:], out_offset=None, in_=uemb_in[:],
                in_offset=bass.IndirectOffsetOnAxis(ap=upad_sb[:, :1], axis=0),
                bounds_check=NUSR - 1, oob_is_err=False).then_inc(sG, 16)
            gpsimd.wait_ge(vq, MK['moff'])
            for j in range(TOPK):
                gpsimd.indirect_dma_start(
                    out=g_sb[:, j * 3 * BLK:(j + 1) * 3 * BLK], out_offset=None,
                    in_=packed_in[:],
                    in_offset=bass.IndirectOffsetOnAxis(ap=moff_sb[:, j:j + 1], axis=0),
                    bounds_check=NBLK - 1, oob_is_err=False).then_inc(sG, 16)
            gpsimd.wait_ge(vq, MK['dstoff'])
            for j in range(TOPK):
                gpsimd.indirect_dma_start(
                    out=paug_sb[:, j * (DIM + 1):j * (DIM + 1) + DIM], out_offset=None,
                    in_=poi_in[:],
                    in_offset=bass.IndirectOffsetOnAxis(ap=dstoff_sb[:, j:j + 1], axis=0),
                    bounds_check=NPOI - 1, oob_is_err=False).then_inc(sG, 16)
            gpsimd.wait_ge(sCCI, 16)
            gpsimd.collective_compute(
                "AllGather", mybir.AluOpType.bypass,
                replica_groups=[list(range(NCORES))],
                ins=[cc_in[:]], outs=[cc_ag[:]]).then_inc(sCC, 1)

        @block.tensor
        def _(tensor):
            tensor.wait_ge(sG, 32)            # u2 gathered
            tensor.wait_ge(vq, MK['setup'])   # ones11
            nc.tensor.transpose(out=psum_t1[:], in_=u2_sb[0:1, :],
                                identity=ones11_sb[:]).then_inc(sPE, 1)
            tensor.wait_ge(vq, MK['ucol'])
            tensor.wait_ge(sC, 80)
            nc.tensor.matmul(out=psum_fc1[:], lhsT=ucol_sb[:], rhs=fcw1_sb[:],
                             start=True, stop=True).then_inc(sPE, 1)
            tensor.wait_ge(vq, MK['dstoff'])
            tensor.wait_ge(sG, 96)            # poi gathered
            for j in range(TOPK):
                mmx = nc.tensor.matmul(
                    out=psum_nd[:], lhsT=wf_sb[:, j:j + 1],
                    rhs=paug_sb[:, j * (DIM + 1):(j + 1) * (DIM + 1)],
                    start=(j == 0), stop=(j == TOPK - 1))
            mmx.then_inc(sPE, 1)
            tensor.wait_ge(sRED, 16)
            nc.tensor.matmul(out=psum_nc[:], lhsT=cc8_sb[:, 0:DIM], rhs=ones8_sb[:],
                             start=True, stop=True)
            nc.tensor.matmul(out=psum_den[:], lhsT=cc8_sb[:, DIM:DIM + 1],
                             rhs=ones8_sb[:], start=True, stop=True).then_inc(sPE, 1)
            tensor.wait_ge(vq, MK['ncol'])
            nc.tensor.matmul(out=psum_fc2[:], lhsT=ncol_sb[:], rhs=fcw2_sb[:],
                             start=True, stop=True).then_inc(sPE, 1)

    es.close()
    return nc


_BUILT = None


def _get_nc():
    global _BUILT
    if _BUILT is None:
        _BUILT = _build()
    return _BUILT


_BLKIO = None


def _make_in_maps(inputs):
    global _BLKIO
    user_idx = np.asarray(inputs["user_idx"]).astype(np.int32)
    poi = np.ascontiguousarray(np.asarray(inputs["poi_embeddings"], dtype=np.float32))
    src = np.asarray(inputs["edge_src"]).astype(np.int32)
    dst = np.asarray(inputs["edge_dst"]).astype(np.int32)
    freq = np.asarray(inputs["edge_freq"]).astype(np.int32)
    uemb = np.ascontiguousarray(np.asarray(inputs["user_emb"], dtype=np.float32))
    fc_w = np.asarray(inputs["fc_w"], dtype=np.float32)
    fc_b = np.asarray(inputs["fc_b"], dtype=np.float32)

    uid = int(user_idx[0])
    uidrep = np.full((P, 1), float(uid), np.float32)
    uidpad = np.full((2, 1), uid, np.int32)
    fcwt = np.ascontiguousarray(fc_w.T)
    fcb = fc_b.reshape(1, DIM)
    if _BLKIO is None:
        _BLKIO = (np.arange(P * NBLKF, dtype=np.float32) + 1.0).reshape(P, NBLKF)

    # safety: the static graph extracts at most TOPK matched blocks and TOPK
    # matched edges per partition; verify the actual data fits (fail loudly
    # rather than return a wrong answer).
    m = src == uid
    mpart = m.reshape(NCORES * P, FREE)
    epp = mpart.sum(1)
    bpp = mpart.reshape(NCORES * P, NBLKF, BLK).any(2).sum(1)
    assert epp.max() <= TOPK, f"edges/partition {epp.max()} > {TOPK}"
    assert bpp.max() <= TOPK, f"blocks/partition {bpp.max()} > {TOPK}"
    packs = (dst.astype(np.int64) * 64 + freq).reshape(NCORES * P, FREE)
    for prow in np.nonzero(epp > 1)[0]:
        vals = packs[prow][mpart[prow]]
        assert len(set(vals.tolist())) == len(vals), "duplicate (dst,freq) in partition"

    in_maps = []
    for c in range(NCORES):
        sl = slice(c * ESH, (c + 1) * ESH)
        packed = np.concatenate(
            [src[sl].reshape(NBLK, BLK), dst[sl].reshape(NBLK, BLK),
             freq[sl].reshape(NBLK, BLK)], axis=1)
        in_maps.append({
            "src": np.ascontiguousarray(src[sl].reshape(P, FREE)),
            "packed": np.ascontiguousarray(packed),
            "uidrep": uidrep, "uidpad": uidpad, "blkio": _BLKIO,
            "poi": poi, "uemb": uemb, "fcwt": fcwt, "fcb": fcb,
        })
    return in_maps


def kernel(**inputs):
    from concourse.bass_utils import run_bass_kernel_spmd

    in_maps = _make_in_maps(inputs)
    nc = _get_nc()
    res = run_bass_kernel_spmd(nc, in_maps, list(range(NCORES)))
    return np.asarray(res.results[0]["out"], dtype=np.float32)
